# revision 14
# baseline (speedup 1.0000x reference)
"""GPTBigCode MQA attention layer on 8 TRN2 NeuronCores.

Sharding: data-parallel over batch (2) x tensor-parallel over head groups
(4 groups of 4 heads).  Core = (b, g).  Each core computes:
  qkvT = W_qkv[:, cols].T @ X[b].T        (cols = Q cols of group + shared KV)
  per head: scoresT = K^T.T @ Q^T, P = exp(scale*scoresT) (causal, no max-sub),
  attnT = V.T @ P, col-sums via ones-matmul, scale by 1/sum,
  partialT = W_proj[rows].T @ attnT
Host gathers: out[b] = sum_g partialT.T + c_proj_b.

v2: QKV and c_proj matmuls run in fp8e4 DoubleRow mode (0.5 cyc/row over
256-deep contraction pairs = 4x bf16 throughput).  Full bf16 precision is
retained by an exact hi/lo decomposition: x = hi + lo with hi = fp8(x),
lo = fp8(x - hi), and W.T@X = Wh.T@Xh + Wh.T@Xl + Wl.T@Xh (the lo.lo term
is ~2^-10 relative and dropped), i.e. 24 DoubleRow passes replace 16 bf16
passes per QKV chain: 0.75x PE cycles.  fp8e4's narrow exponent range is
dodged by pow2 prescales folded into existing constants (nothing extra on
any engine): X*16 and W_qkv*64 on host -> QKV PSUM at 2^10; Q,K,V copies
stay plain; exp scale absorbs 2^-20; ones tile = 64 absorbs V's 2^10 so
attnT lands at 16x true scale (fp8-safe); W_proj*64 on host -> c_proj PSUM
at 2^10, descaled in the existing output copies.  attnT is split hi/lo on
device (one Act copy + one DVE sub per head-macro).  Scores/PV stay bf16:
their contraction per score tile is a single 128 (D) so DoubleRow pairing
cannot beat 1.0 cyc/row with the exact 3-term split.

Schedule notes (inherited from the 215us bf16 design):
  - software pipeline: per head, scores+exps first with the PV burst one
    head behind; last head of each macro deferred across the next macro's
    QKV window; c_proj for macro m interleaved into macro m+1's attention.
  - scores land in f32 [128,2,512] PSUM pair tiles; causal diagonal
    k-tiles compute only q >= j*128; one [128,128] triangle mask.
  - softmax denominators off the PE: DVE accumulates exp tiles in fp16,
    one closing ones-matmul per head.
  - QKV t-outer with 3 concurrent chains tracking the startup DMA stream;
    W_qkv host-permuted to [K, V, Q0..3] so pass A needs only cols 0:384.
  - V^T -> V transpose on the DMA xbar.
  - input DMAs batched via 3D APs; hi stream first (feeds the hi.hi
    passes), lo stream behind it; xt prefetched for all macros; outputs
    staged bf16 [128,4,512] per group.
"""

import sys

sys.path.insert(0, "/opt/trn_rl_repo")

import numpy as np
import ml_dtypes

S = 2048
E = 2048
D = 128
HG = 4  # heads per core
SM = 4  # s macro-tiles of 512
ET = 16  # e (contraction) tiles of 128
# net softmax input scale; 2^-20 descales the Q,K pow2 prescale (16*64)^2
SCALE = (1.0 / float(np.sqrt(D))) / float(2**20)

_CACHE = {}


def _build_nc(with_bias=False):
    import concourse.bacc as bacc
    import concourse.mybir as mybir
    import concourse.tile as tile
    from concourse.bass import ds, ts
    from contextlib import ExitStack

    BF = mybir.dt.bfloat16
    F32 = mybir.dt.float32
    F16 = mybir.dt.float16
    F8 = mybir.dt.float8e4
    Act = mybir.ActivationFunctionType
    PM = mybir.MatmulPerfMode.DoubleRow

    nc = bacc.Bacc("TRN2", target_bir_lowering=False, debug=False)

    xh_d = nc.dram_tensor("xt_hi", (E, S), F8, kind="ExternalInput")
    xl_d = nc.dram_tensor("xt_lo", (E, S), F8, kind="ExternalInput")
    wqh_d = nc.dram_tensor("wq_hi", (E, 768), F8, kind="ExternalInput")
    wql_d = nc.dram_tensor("wq_lo", (E, 768), F8, kind="ExternalInput")
    bq_d = (
        nc.dram_tensor("b_qkv", (768, 1), F32, kind="ExternalInput")
        if with_bias
        else None
    )
    wph_d = nc.dram_tensor("wp_hi", (512, E), F8, kind="ExternalInput")
    wpl_d = nc.dram_tensor("wp_lo", (512, E), F8, kind="ExternalInput")
    mk_d = nc.dram_tensor("maskt", (128, 128), BF, kind="ExternalInput")
    out_d = nc.dram_tensor("outT", (E, S), BF, kind="ExternalOutput")

    with ExitStack() as ctx:
        tc = ctx.enter_context(tile.TileContext(nc))
        const = ctx.enter_context(tc.tile_pool(name="const", bufs=1))
        xpool = ctx.enter_context(tc.tile_pool(name="xpool", bufs=2))
        ptpool = ctx.enter_context(tc.tile_pool(name="ptpool", bufs=32))
        rpool = ctx.enter_context(tc.tile_pool(name="rpool", bufs=2))
        abpool = ctx.enter_context(tc.tile_pool(name="abpool", bufs=2))
        spool = ctx.enter_context(tc.tile_pool(name="spool", bufs=2))
        opool = ctx.enter_context(tc.tile_pool(name="opool", bufs=3))
        psA = ctx.enter_context(tc.tile_pool(name="psA", bufs=2, space="PSUM"))
        scp = ctx.enter_context(tc.tile_pool(name="scp", bufs=2, space="PSUM"))
        psC = ctx.enter_context(tc.tile_pool(name="psC", bufs=2, space="PSUM"))

        # --- persistent SBUF tensors ---
        wqh_sb = const.tile([128, ET, 768], F8, tag="wqh")  # W_qkv hi, e-tile major
        wql_sb = const.tile([128, ET, 768], F8, tag="wql")  # W_qkv lo
        wph_sb = const.tile([128, 4, E], F8, tag="wph")  # W_proj hi, c-tile major
        wpl_sb = const.tile([128, 4, E], F8, tag="wpl")  # W_proj lo
        mk_sb = const.tile([128, 128], BF, tag="mk")  # causal triangle 0/1
        bq_sb = const.tile([128, 6], F32, tag="bq")  # qkv bias per c-tile
        qk_sb = const.tile([128, 5, S], BF, tag="qk")  # Q^T (4 heads) + K^T
        vt_sb = const.tile([128, S], BF, tag="vt")  # V^T staging
        v_sb = const.tile([128, ET, D], BF, tag="v")  # V, k-tile major
        ath_sb = const.tile([128, HG, S], F8, tag="ath")  # attnT fp8 hi
        atl_sb = const.tile([128, HG, S], F8, tag="atl")  # attnT fp8 lo
        ones_h = const.tile([128, 1], F16, tag="ones_h")

        def _dram_grp(dram, t0, n, c0, w):
            return dram[ds(t0 * 128, n * 128), ds(c0, w)].rearrange(
                "(t p) c -> p t c", p=128
            )

        # --- DMAs.  The cost model charges ~500-630ns PER ISSUE with the
        # transfer time (per-partition bytes / 38 B/ns) essentially free at
        # our sizes, so: as few issues as possible, graduated only at the
        # very front so the first DoubleRow pair can start ~1.8us in.
        # W_qkv columns are host-permuted to [K, V, Q0..Q3] so pass A
        # (K, V, Q0) needs only wq cols 0:384.
        if with_bias:
            for c in range(6):
                nc.scalar.dma_start(
                    out=bq_sb[:, c : c + 1], in_=bq_d[ts(c, 128), :]
                )
        xhs, xls = [], []
        for m in range(SM):
            xhs.append(xpool.tile([128, ET, 512], F8, tag="xh", name=f"xh{m}"))
            xls.append(xpool.tile([128, ET, 512], F8, tag="xl", name=f"xl{m}"))

        # first pair on each queue feeds DoubleRow pass j=0 of half A
        nc.scalar.dma_start(
            out=wqh_sb[:, ds(0, 2), ds(0, 384)],
            in_=_dram_grp(wqh_d, 0, 2, 0, 384),
        )
        nc.sync.dma_start(
            out=xhs[0][:, ds(0, 2), :], in_=_dram_grp(xh_d, 0, 2, 0, 512)
        )
        nc.scalar.dma_start(out=mk_sb, in_=mk_d[:, :])
        # ones=64 folds V's 2^10 prescale: recip = 1/(64*sum) so the
        # normalize lands attnT at 16x true scale (fp8-safe range).
        nc.vector.memset(ones_h, 64.0)
        # rest of macro 0 (hi then lo), then everything else, few big DMAs
        nc.sync.dma_start(
            out=xhs[0][:, ds(2, 14), :], in_=_dram_grp(xh_d, 2, 14, 0, 512)
        )
        nc.scalar.dma_start(
            out=wqh_sb[:, ds(2, 14), ds(0, 384)],
            in_=_dram_grp(wqh_d, 2, 14, 0, 384),
        )
        nc.sync.dma_start(out=xls[0], in_=_dram_grp(xl_d, 0, ET, 0, 512))
        nc.scalar.dma_start(
            out=wql_sb[:, :, ds(0, 384)], in_=_dram_grp(wql_d, 0, ET, 0, 384)
        )
        nc.scalar.dma_start(
            out=wqh_sb[:, :, ds(384, 384)], in_=_dram_grp(wqh_d, 0, ET, 384, 384)
        )
        nc.scalar.dma_start(
            out=wql_sb[:, :, ds(384, 384)], in_=_dram_grp(wql_d, 0, ET, 384, 384)
        )
        # macros 1..3, one issue per stream per macro (ring-buffered); wp
        # after xt/wq
        for m in range(1, SM):
            nc.sync.dma_start(
                out=xhs[m], in_=_dram_grp(xh_d, 0, ET, m * 512, 512)
            )
            nc.sync.dma_start(
                out=xls[m], in_=_dram_grp(xl_d, 0, ET, m * 512, 512)
            )
        nc.scalar.dma_start(out=wph_sb, in_=_dram_grp(wph_d, 0, 4, 0, E))
        nc.scalar.dma_start(out=wpl_sb, in_=_dram_grp(wpl_d, 0, 4, 0, E))

        def _qkv(m):
            # QKV projection, fp8 3-term DoubleRow, t-outer with 3
            # concurrent chains.  c-block order: 0=K, 1=V, 2..5=Q heads.
            sm = ds(m * 512, 512)

            def _qkv_dest(c):
                if c == 0:
                    return qk_sb[:, 4, sm]
                if c == 1:
                    return vt_sb[:, sm]
                return qk_sb[:, c - 2, sm]

            terms = ((wqh_sb, xhs[m]), (wqh_sb, xls[m]), (wql_sb, xhs[m]))
            for half in range(2):
                cs = [3 * half + i for i in range(3)]
                chains = [
                    psA.tile([128, 512], F32, tag="mmA", name=f"q{m}{c}")
                    for c in cs[:2]
                ]
                chains.append(
                    scp.tile([128, 2, 512], F32, tag="sc", name=f"q{m}x")[:, 0, :]
                )
                for ti, (wsb, xsb) in enumerate(terms):
                    for j in range(8):
                        for ci, c in enumerate(cs):
                            nc.tensor.matmul(
                                chains[ci],
                                lhsT=wsb[:, ds(2 * j, 2), ds(c * 128, 128)],
                                rhs=xsb[:, ds(2 * j, 2), :],
                                start=(ti == 0 and j == 0),
                                stop=(ti == 2 and j == 7),
                                perf_mode=PM,
                            )
                # scp-hosted chain copied first: the scores sc ring waits on
                # its release, so this unblocks attention ~1.2us earlier
                for ci in (2, 0, 1):
                    c = cs[ci]
                    if with_bias:
                        nc.vector.tensor_scalar_add(
                            _qkv_dest(c), chains[ci], bq_sb[:, c : c + 1]
                        )
                    else:
                        nc.vector.tensor_copy(
                            out=_qkv_dest(c), in_=chains[ci]
                        )
                if half == 0:
                    # V^T ready after pass 0 (c-blocks K, V): transpose the
                    # whole macro slice on the DMA xbar, no PE/PSUM needed.
                    nc.sync.dma_start_transpose(
                        out=v_sb[:, ds(4 * m, 4), :], in_=vt_sb[:, sm]
                    )

        def _scores(m, h):
            # scores land in fp32 PSUM pair tiles (two k-tiles share a
            # bank): doubles the PE lookahead the PSUM ring allows and
            # halves the exp instruction count for full tiles.
            nkt = 4 * (m + 1)
            pts = []
            kt = 0
            while kt < nkt:
                j = kt - 4 * m
                if j < 0 and kt + 1 < 4 * m:
                    sc = scp.tile([128, 2, 512], F32, tag="sc")
                    ptp = ptpool.tile(
                        [128, 2, 512], BF, tag="pt", name=f"p{m}{h}"
                    )
                    for i in (0, 1):
                        nc.tensor.matmul(
                            sc[:, i, :],
                            lhsT=qk_sb[:, 4, ds((kt + i) * 128, 128)],
                            rhs=qk_sb[:, h, ds(m * 512, 512)],
                            start=True,
                            stop=True,
                        )
                    nc.scalar.activation(
                        out=ptp, in_=sc, func=Act.Exp, bias=0.0, scale=SCALE
                    )
                    pts.append((ptp[:, 0, :], 0, 512))
                    pts.append((ptp[:, 1, :], 0, 512))
                    kt += 2
                else:
                    off = j * 128 if j > 0 else 0
                    w = 512 - off
                    sc = scp.tile([128, 2, 512], F32, tag="sc")
                    ptp = ptpool.tile(
                        [128, 2, 512], BF, tag="pt", name=f"p{m}{h}"
                    )
                    nc.tensor.matmul(
                        sc[:, 0, ds(0, w)],
                        lhsT=qk_sb[:, 4, ds(kt * 128, 128)],
                        rhs=qk_sb[:, h, ds(m * 512 + off, w)],
                        start=True,
                        stop=True,
                    )
                    nc.scalar.activation(
                        out=ptp[:, 0, ds(off, w)],
                        in_=sc[:, 0, ds(0, w)],
                        func=Act.Exp,
                        bias=0.0,
                        scale=SCALE,
                    )
                    if j >= 0:
                        nc.vector.tensor_mul(
                            ptp[:, 0, ds(off, 128)],
                            ptp[:, 0, ds(off, 128)],
                            mk_sb,
                        )
                    pts.append((ptp[:, 0, :], off, w))
                    kt += 1
            return pts

        def _sums(m, h, pts):
            # DVE accumulates the tail tiles in fp16; the first two are
            # fused into one add (seed tile is always full width).
            nkt = 4 * (m + 1)
            sp = spool.tile([128, 512], F16, tag="spacc")
            pt0, off0, w0 = pts[0]
            pt1, off1, w1 = pts[1]
            if off1:
                nc.vector.tensor_copy(
                    out=sp[:, ds(0, off1)], in_=pt0[:, ds(0, off1)]
                )
            nc.vector.tensor_add(
                sp[:, ds(off1, w1)], pt0[:, ds(off1, w1)], pt1[:, ds(off1, w1)]
            )
            for i in range(2, nkt):
                pt, off, w = pts[i]
                nc.vector.tensor_add(
                    sp[:, ds(off, w)], sp[:, ds(off, w)], pt[:, ds(off, w)]
                )
            return sp

        def _pv_mms(m, pts):
            nkt = 4 * (m + 1)
            ps_at = psC.tile([128, 512], F32, tag="attnacc", bufs=1)
            for kt in range(nkt):
                pt, off, w = pts[kt]
                nc.tensor.matmul(
                    ps_at[:, ds(off, w)],
                    lhsT=v_sb[:, kt, :],
                    rhs=pt[:, ds(off, w)],
                    start=(kt == 0),
                    stop=(kt == nkt - 1),
                )
            return ps_at

        def _pv_close(m, h, ps_at, sp):
            sm = ds(m * 512, 512)
            ps_sum = psC.tile([1, 512], F32, tag="sum", bufs=1)
            nc.tensor.matmul(
                ps_sum, lhsT=ones_h, rhs=sp, start=True, stop=True
            )
            recip = rpool.tile([1, 512], F32, tag="recip")
            nc.vector.reciprocal(recip, ps_sum)
            bc_sb = rpool.tile([128, 512], F32, tag="bc")
            nc.gpsimd.partition_broadcast(bc_sb, recip)
            at_bf = abpool.tile([128, 512], BF, tag="atb")
            nc.vector.tensor_mul(at_bf, ps_at, bc_sb)
            # fp8 hi/lo split of attnT for the DoubleRow c_proj
            nc.scalar.activation(
                out=ath_sb[:, h, sm],
                in_=at_bf,
                func=Act.Copy,
                bias=0.0,
                scale=1.0,
            )
            nc.vector.tensor_sub(atl_sb[:, h, sm], at_bf, ath_sb[:, h, sm])

        def _pv(m, h, pts, sp):
            _pv_close(m, h, _pv_mms(m, pts), sp)

        def _attention(m, cproj_m=None, pend=None):
            prev = pend
            for h in range(HG):
                if prev is not None:
                    pm_, ph, pp = prev
                    sp = _sums(pm_, ph, pp)
                pts = _scores(m, h)
                if prev is not None:
                    _pv(pm_, ph, pp, sp)
                if cproj_m is not None:
                    _cproj_group(cproj_m, h)
                prev = (m, h, pts)
            return prev

        def _cproj_mms(ps_o, eo, sm, only_cj=None):
            # 3-term fp8 DoubleRow over head pairs (0,1),(2,3)
            terms = ((wph_sb, ath_sb), (wph_sb, atl_sb), (wpl_sb, ath_sb))
            cjs = (0, 1) if only_cj is None else (only_cj,)
            for ti, (wsb, asb) in enumerate(terms):
                for cj in cjs:
                    nc.tensor.matmul(
                        ps_o,
                        lhsT=wsb[:, ds(2 * cj, 2), ds(eo * 128, 128)],
                        rhs=asb[:, ds(2 * cj, 2), sm],
                        start=(ti == 0 and cj == cjs[0] and only_cj != 1),
                        stop=(ti == 2 and cj == cjs[-1] and only_cj != 0),
                        perf_mode=PM,
                    )

        def _ob_copy(ob_slot, ps_o, use_act):
            if use_act:
                nc.scalar.activation(
                    out=ob_slot,
                    in_=ps_o,
                    func=Act.Copy,
                    bias=0.0,
                    scale=2.0**-10,
                )
            else:
                nc.vector.tensor_scalar_mul(ob_slot, ps_o, 2.0**-10)

        def _cproj_group(m, g, act_only=False):
            # one c_proj output group (4 eo blocks) for s-macro m
            sm = ds(m * 512, 512)
            ob = opool.tile([128, 4, 512], BF, tag="ob")
            for i in range(4):
                eo = g * 4 + i
                ps_o = psA.tile([128, 512], F32, tag="mmA", name=f"o{m}{eo}")
                _cproj_mms(ps_o, eo, sm)
                _ob_copy(ob[:, i, :], ps_o, act_only or i % 2 == 1)
            q = nc.scalar if g % 2 else nc.sync
            q.dma_start(
                out=out_d[ds(g * 512, 512), sm].rearrange(
                    "(i p) c -> p i c", p=128
                ),
                in_=ob,
            )

        # software pipeline: c_proj for macro m is interleaved into macro
        # m+1's attention bursts, and the last head of each macro is
        # deferred across the next macro's QKV window, so its exp/sum
        # chains drain while the PE runs QKV.
        _qkv(0)
        pend = _attention(0)
        for m in range(1, SM - 1):
            _qkv(m)
            pend = _attention(m, cproj_m=m - 1, pend=pend)

        # --- final macro: attention with the last head NOT deferred.  Its
        # sums go to DVE before the cproj(2,3) copies so the 15-add chain
        # drains under PE work, and the c_proj for the final macro opens
        # six chains on the (0,1) head pairs to bridge the last normalize
        # chain's latency.
        mF = SM - 1
        smF = ds(mF * 512, 512)
        _qkv(mF)
        prev = pend
        for h in range(HG - 1):
            pm_, ph, pp = prev
            sp = _sums(pm_, ph, pp)
            pts = _scores(mF, h)
            _pv(pm_, ph, pp, sp)
            _cproj_group(mF - 1, h)
            prev = (mF, h, pts)
        pm_, ph, pp = prev
        sp = _sums(pm_, ph, pp)
        pts3 = _scores(mF, HG - 1)
        _pv(pm_, ph, pp, sp)
        sp3 = _sums(mF, HG - 1, pts3)  # DVE starts the 15-add chain now
        _cproj_group(mF - 1, HG - 1, act_only=True)  # keep DVE clear
        ps_at3 = _pv_mms(mF, pts3)
        # pre-open six c_proj chains (eo 0..5) on the ready (0,1) head
        # pairs: 2 psA banks + both halves of 2 scp tiles; split around
        # the ones-matmul so the PE stays fed while sums3 drains.
        scpa = scp.tile([128, 2, 512], F32, tag="sc", name="fo_a")
        scpb = scp.tile([128, 2, 512], F32, tag="sc", name="fo_b")
        chain_ps = [
            psA.tile([128, 512], F32, tag="mmA", name="fo0"),
            psA.tile([128, 512], F32, tag="mmA", name="fo1"),
            scpa[:, 0, :],
            scpa[:, 1, :],
            scpb[:, 0, :],
            scpb[:, 1, :],
        ]
        for eo in (0, 1, 2):
            _cproj_mms(chain_ps[eo], eo, smF, only_cj=0)
        _pv_close(mF, HG - 1, ps_at3, sp3)
        for eo in (3, 4, 5):
            _cproj_mms(chain_ps[eo], eo, smF, only_cj=0)
        # close the six opened chains, then run eo 6..15 start-to-finish
        ob0 = opool.tile([128, 4, 512], BF, tag="ob", name="fob0")
        ob1 = opool.tile([128, 4, 512], BF, tag="ob", name="fob1")
        for eo in range(6):
            _cproj_mms(chain_ps[eo], eo, smF, only_cj=1)
            ob, i = (ob0, eo) if eo < 4 else (ob1, eo - 4)
            _ob_copy(ob[:, i, :], chain_ps[eo], eo % 2 == 1)
            if eo == 3:
                nc.sync.dma_start(
                    out=out_d[ds(0, 512), smF].rearrange(
                        "(i p) c -> p i c", p=128
                    ),
                    in_=ob0,
                )
        for eo in range(6, 16):
            g, i = eo // 4, eo % 4
            if eo == 8:
                ob = opool.tile([128, 4, 512], BF, tag="ob", name="fob2")
            elif eo == 12:
                ob = opool.tile([128, 4, 512], BF, tag="ob", name="fob3")
            elif eo < 8:
                ob = ob1
            ps_o = psA.tile([128, 512], F32, tag="mmA", name=f"fo{eo}")
            _cproj_mms(ps_o, eo, smF)
            _ob_copy(ob[:, i, :], ps_o, eo % 2 == 1)
            if g == 3:
                # final group: per-eo DMAs so the tail is one eo deep
                q = nc.scalar if eo % 2 else nc.sync
                q.dma_start(out=out_d[ds(eo * 128, 128), smF], in_=ob[:, i, :])
            elif i == 3:
                q = nc.scalar if g % 2 else nc.sync
                q.dma_start(
                    out=out_d[ds(g * 512, 512), smF].rearrange(
                        "(i p) c -> p i c", p=128
                    ),
                    in_=ob,
                )

    nc.compile()
    return nc


def _get_nc(with_bias=False):
    key = f"nc{with_bias}"
    if key not in _CACHE:
        _CACHE[key] = _build_nc(with_bias)
    return _CACHE[key]


def _host_mask():
    k = np.arange(128)[:, None]
    q = np.arange(128)[None, :]
    return (k <= q).astype(ml_dtypes.bfloat16)


def _split8(a):
    """Exact-ish fp8 hi/lo split: a ~= hi + lo elementwise."""
    f8 = ml_dtypes.float8_e4m3
    hi = a.astype(f8)
    lo = (a - hi.astype(np.float32)).astype(f8)
    return hi, lo


def kernel(**inputs):
    from concourse.bass_utils import run_bass_kernel_spmd

    hidden = np.asarray(inputs["hidden_states"], dtype=np.float32)
    caw = np.asarray(inputs["c_attn_w"], dtype=np.float32)
    cab = np.asarray(inputs["c_attn_b"], dtype=np.float32)
    cpw = np.asarray(inputs["c_proj_w"], dtype=np.float32)
    cpb = np.asarray(inputs["c_proj_b"], dtype=np.float32)

    maskb = _host_mask()
    # pow2 prescales keep fp8e4's exponent range happy; all descale folds
    # into existing kernel constants (see module docstring).
    xt_by_batch = [_split8(hidden[b].T * 16.0) for b in range(2)]
    in_maps = []
    for core in range(8):
        b, g = core % 2, core // 2
        # column order [K, V, Q-group]: pass A of the QKV projection only
        # needs the first 384 columns, shrinking the startup DMA wave.
        cols = np.r_[E : E + D, E + D : E + 2 * D, g * 512 : (g + 1) * 512]
        wqh, wql = _split8(caw[:, cols] * 64.0)
        wph, wpl = _split8(cpw[g * 512 : (g + 1) * 512, :] * 64.0)
        in_maps.append(
            {
                "xt_hi": xt_by_batch[b][0],
                "xt_lo": xt_by_batch[b][1],
                "wq_hi": wqh,
                "wq_lo": wql,
                "b_qkv": (cab[cols] * 1024.0).reshape(768, 1).astype(np.float32),
                "wp_hi": wph,
                "wp_lo": wpl,
                "maskt": maskb,
            }
        )

    with_bias = bool(np.any(cab))
    if not with_bias:
        for im in in_maps:
            del im["b_qkv"]
    nc = _get_nc(with_bias=with_bias)
    res = run_bass_kernel_spmd(nc, in_maps, core_ids=list(range(8)))
    out = np.zeros((2, S, E), np.float32)
    for core in range(8):
        b = core % 2
        out[b] += res.results[core]["outT"].T.astype(np.float32)
    out += cpb[None, None, :]
    return out
